# revision 24
# baseline (speedup 1.0000x reference)
"""Trainium2 Bass kernel for CrossModalAttention2D — channel-sharded variant.

Same math as the batch-parallel version (see kernel docstring history):
out = visual + gamma * p[b],  p = ((t Wv^T + bv) Wo^T + bo) Wp^T + bp.

Sharding: core c handles channel block [128c, 128(c+1)) of ALL 8 batches.
The chain's first two layers (full 1024-wide outputs) are computed on
every core with M=8 batch lanes, but layer 3 only needs the core's 128
output rows of proj_w — the 1 MB fp8 Wp shrinks to 128 KB per core,
cutting per-core HBM traffic from 19.05 to 18.2 MB.
"""

import os
import sys

sys.path.insert(0, "/opt/trn_rl_repo")

import numpy as np

import concourse.bass as bass
import concourse.mybir as mybir
from concourse.tile import TileContext
from concourse.bass_utils import run_bass_kernel_spmd

B, C, H, W, NH, NT = 8, 1024, 64, 64, 16, 8
HW = H * W
P = 128
NCH = C // P
F32 = mybir.dt.float32
F16 = mybir.dt.float16
BF16 = mybir.dt.bfloat16
F8 = mybir.dt.float8e4
WDT = F8
PADW = 16
HALF = C // 2
HC = NCH * C // 2      # half of a full weight matrix's SBUF cols
HCB = NCH * P // 2     # half of the Wp-block's SBUF cols

# textblob (fp16): tta (NCH*B*NT=512 cols) — rides SWDGE's first push so
# the 128 KB stays off the HWDGE heads (weights there instead).
# colblob (fp32, 13 KB): [ bvc (8) | boc (8) | bpb (1) | gamma (1) |
#                          ident8 (8, rows 0-7) ]
TTA = NCH * B * NT
CBW = 2 * NCH + 2 + 8


def _split_waits(nc):
    for fn in nc.m.functions:
        for blk in fn.blocks:
            rebuilt = []
            changed = False
            for inst in blk.instructions:
                si = inst.sync_info
                if si is not None and si.on_wait is not None and len(si.on_wait) > 1:
                    waits = list(si.on_wait)
                    for i, w in enumerate(waits[:-1]):
                        rebuilt.append(
                            mybir.InstNoOp(
                                name=f"{inst.name}-ws{i}",
                                engine=inst.engine,
                                sync_info=mybir.SyncInfo(on_wait=[w], on_update=[]),
                                bass_nofuse=True,
                            )
                        )
                    si.on_wait = [waits[-1]]
                    changed = True
                rebuilt.append(inst)
            if changed:
                blk.instructions = rebuilt


def _build_nc():
    nc = bass.Bass(trn_type="TRN2")

    vis = nc.dram_tensor("vis", [B * P, HW], F16, kind="ExternalInput")
    wv_sb = nc.dram_tensor("wv_sb", [P, NCH * C], WDT, kind="ExternalInput")
    wo_sb = nc.dram_tensor("wo_sb", [P, NCH * C], WDT, kind="ExternalInput")
    wpb_sb = nc.dram_tensor("wpb_sb", [P, NCH * P], WDT, kind="ExternalInput")
    textblob = nc.dram_tensor("textblob", [P, TTA], F16, kind="ExternalInput")
    colblob = nc.dram_tensor("colblob", [P, CBW], F32, kind="ExternalInput")
    out = nc.dram_tensor("out", [B * P, HW], F16, kind="ExternalOutput")

    with TileContext(nc) as tc:
        with (
            tc.tile_pool(name="cols", bufs=1) as cols,
            tc.tile_pool(name="wpool", bufs=6) as wpool,
            tc.tile_pool(name="psum", bufs=NCH, space="PSUM") as psum,
            tc.tile_pool(name="vispool", bufs=B) as vispool,
        ):
            # ---- weight halves lead both HWDGE queues (per-queue delivery
            # is ~1/3 fabric rate; head position = early arrival) ----
            wts = {
                "v": [wpool.tile([P, HC], WDT, tag=f"wtv{h}", bufs=1,
                                 name=f"wtv{h}") for h in range(2)],
                "u": [wpool.tile([P, HC], WDT, tag=f"wtu{h}", bufs=1,
                                 name=f"wtu{h}") for h in range(2)],
                "g": [wpool.tile([P, HCB], WDT, tag=f"wtg{h}", bufs=1,
                                 name=f"wtg{h}") for h in range(2)],
            }
            nc.sync.dma_start(out=wts["v"][0], in_=wv_sb[:, :HC])
            nc.sync.dma_start(out=wts["u"][0], in_=wo_sb[:, :HC])
            nc.sync.dma_start(out=wts["g"][0], in_=wpb_sb[:, :HCB])
            cb = cols.tile([P, CBW], F32, tag="cb")
            nc.scalar.dma_start(out=cb, in_=colblob[:, :])
            nc.scalar.dma_start(out=wts["v"][1], in_=wv_sb[:, HC:])
            nc.scalar.dma_start(out=wts["u"][1], in_=wo_sb[:, HC:])
            nc.scalar.dma_start(out=wts["g"][1], in_=wpb_sb[:, HCB:])
            tbl = cols.tile([P, TTA], F16, tag="tbl")
            nc.gpsimd.dma_start(out=tbl, in_=textblob[:, :])

            tta = tbl.rearrange("p (k b n) -> p (k b) n", k=NCH, b=B, n=NT)
            bvc = cb[:, 0:NCH]
            boc = cb[:, NCH : 2 * NCH]
            bpb = cb[:, 2 * NCH : 2 * NCH + 1]
            gc = cb[:, 2 * NCH + 1 : 2 * NCH + 2]
            id8f = cb[0:8, 2 * NCH + 2 : CBW]
            id8b = cols.tile([8, 8], BF16, tag="id8b")
            nc.vector.tensor_copy(id8b, id8f)

            vts = {m: vispool.tile([P, HW], F16, tag="vt", name=f"vt{m}")
                   for m in range(B)}

            def vload(eng, m):
                eng.dma_start(out=vts[m], in_=vis[m * P : (m + 1) * P, :])

            for m in (4, 5, 6, 7):
                vload(nc.gpsimd, m)
            vload(nc.sync, 0)
            vload(nc.sync, 2)
            vload(nc.scalar, 1)
            vload(nc.scalar, 3)

            # ---- t[b] = mean_n text[b,:,:], all batches, written to the
            # padded stride-16 layout: value (k-chunk, batch) at col 16k+b
            tsum = cols.tile([P, NCH * B], F32, tag="tsum")
            nc.vector.reduce_sum(tsum, tta, axis=mybir.AxisListType.X)
            nc.vector.tensor_scalar_mul(tsum, tsum, 1.0 / NT)
            tb = cols.tile([P, PADW * NCH], WDT, tag="tb")
            nc.vector.tensor_copy(
                tb.rearrange("p (k s) -> p k s", s=PADW)[:, :, 0:B],
                tsum.rearrange("p (k b) -> p k b", b=B))

            # ---- chain with M=8 batch lanes, fp8 DoubleRow ----
            def layer(in_tile, whalves, bias_cols, name):
                last = name == "g"
                FD = P if last else HALF      # rhs free dim per pass
                nh = 1 if last else 2
                psr = [psum.tile([B, FD], F32, tag="ps", name=f"psr_{name}{h}")
                       for h in range(nh)]
                kw = (NCH * P // 2) // P if last else (HC // C)  # k per half
                wks = [wh.rearrange("p (k c) -> p k c", k=NCH // 2)
                       for wh in whalves]
                for kp in range(NCH // 2):
                    lhsT = in_tile[:, 2 * PADW * kp : 2 * PADW * (kp + 1)]\
                        .rearrange("p (two s) -> p two s", two=2)[:, :, 0:B]
                    wk = wks[kp // 2]
                    kl = 2 * (kp % 2)
                    for h in range(nh):
                        nc.tensor.matmul(
                            psr[h], lhsT,
                            wk[:, kl : kl + 2, h * FD : (h + 1) * FD],
                            start=(kp == 0),
                            stop=(kp == NCH // 2 - 1),
                            perf_mode=mybir.MatmulPerfMode.DoubleRow,
                        )
                row = cols.tile([B, P if last else C],
                                F32 if last else BF16, tag=f"row{name}")
                for h in range(nh):
                    nc.vector.tensor_copy(row[:, h * FD : (h + 1) * FD], psr[h])
                if last:
                    pc = psum.tile([P, B], F32, tag="ps", name="psT_g")
                    nc.tensor.transpose(pc, row, id8f)
                    gp_t = cols.tile([P, B], F32, tag="gp")
                    # gp[:, b] = (p_blk + bp_blk) * gamma, all batches at once
                    nc.vector.tensor_scalar(
                        gp_t, pc, bpb[:, 0:1], gc[:, 0:1],
                        op0=mybir.AluOpType.add, op1=mybir.AluOpType.mult,
                    )
                    return gp_t
                out_tile = cols.tile([P, PADW * NCH], WDT, tag=f"oc{name}")
                for mo in range(NCH):
                    pc = psum.tile([P, B], BF16, tag="ps", name=f"psT_{name}{mo}")
                    nc.tensor.transpose(
                        pc, row[:, mo * P : (mo + 1) * P], id8b)
                    nc.vector.tensor_scalar_add(
                        out_tile[:, PADW * mo : PADW * mo + B], pc,
                        bias_cols[:, mo : mo + 1])
                return out_tile

            vtile = layer(tb, wts["v"], bvc, "v")
            utile = layer(vtile, wts["u"], boc, "u")
            gp = layer(utile, wts["g"], None, "g")

            ADD_ORDER = [0, 1, 4, 2, 3, 5, 6, 7]
            for m in ADD_ORDER:
                nc.vector.tensor_scalar_add(vts[m], vts[m], gp[:, m : m + 1])

            for m in (0, 2, 5, 7):
                if m == 7:
                    nc.sync.dma_start(out=out[m * P : (m + 1) * P, : HW // 2],
                                      in_=vts[m][:, : HW // 2])
                else:
                    nc.sync.dma_start(out=out[m * P : (m + 1) * P, :],
                                      in_=vts[m])
            for m in (1, 4, 3, 6, 7):
                if m == 7:
                    nc.scalar.dma_start(out=out[m * P : (m + 1) * P, HW // 2 :],
                                        in_=vts[m][:, HW // 2 :])
                else:
                    nc.scalar.dma_start(out=out[m * P : (m + 1) * P, :],
                                        in_=vts[m])

    _split_waits(nc)
    return nc


def _install_ntff_hook():
    try:
        from antenv.axon_hooks import get_axon_ntff_profile_hook  # noqa: F401
        return
    except ImportError:
        pass
    import contextlib
    import ctypes
    import types

    so_path = "/opt/axon/libaxon_pjrt.so"
    if not os.path.exists(so_path):
        return
    lib = ctypes.CDLL(so_path)
    if not hasattr(lib, "axon_start_nrt_profile"):
        return
    lib.axon_start_nrt_profile.argtypes = [
        ctypes.POINTER(ctypes.c_int64), ctypes.c_size_t,
    ]
    lib.axon_start_nrt_profile.restype = ctypes.c_int64
    lib.axon_stop_nrt_profile.argtypes = [ctypes.c_char_p]
    lib.axon_stop_nrt_profile.restype = ctypes.c_int64

    @contextlib.contextmanager
    def _hook(output_dir, device_ids):
        import jax

        jax.devices()
        if device_ids:
            ids = (ctypes.c_int64 * len(device_ids))(*device_ids)
            rc = lib.axon_start_nrt_profile(ids, len(device_ids))
        else:
            rc = lib.axon_start_nrt_profile(None, 0)
        if rc != 0:
            raise RuntimeError(f"axon_start_nrt_profile rc={rc}")
        try:
            yield
        finally:
            n = lib.axon_stop_nrt_profile(str(output_dir).encode())
            print(f"ntff profile: {n} file(s) written to {output_dir}")

    import antenv

    mod = types.ModuleType("antenv.axon_hooks")
    mod.get_axon_ntff_profile_hook = lambda: _hook
    mod.set_axon_ntff_profile_hook = lambda h: None
    sys.modules["antenv.axon_hooks"] = mod
    antenv.axon_hooks = mod


_NC_CACHE = {}


def _get_nc():
    if "nc" not in _NC_CACHE:
        _NC_CACHE["nc"] = _build_nc()
    return _NC_CACHE["nc"]


def kernel(visual, text, in_proj_w, in_proj_b, out_w, out_b, ln_w, ln_b,
           proj_w, proj_b, gamma):
    visual = np.asarray(visual, dtype=np.float32)
    text = np.asarray(text, dtype=np.float32)
    in_proj_w = np.asarray(in_proj_w, dtype=np.float32)
    in_proj_b = np.asarray(in_proj_b, dtype=np.float32)
    proj_w = np.asarray(proj_w, dtype=np.float32)
    proj_b = np.asarray(proj_b, dtype=np.float32)

    # host-side input marshalling (layout/dtype only, no math)
    import ml_dtypes

    wdt = ml_dtypes.float8_e4m3fn

    def sb_layout(wT, ncols=C):
        return np.ascontiguousarray(
            wT.reshape(NCH, P, ncols).transpose(1, 0, 2).reshape(P, NCH * ncols)
        ).astype(wdt)

    wv_sb = sb_layout(in_proj_w[2 * C : 3 * C].T)
    wo_sb = sb_layout(np.asarray(out_w, dtype=np.float32).T)

    bv_col = in_proj_b[2 * C : 3 * C].reshape(NCH, P).T
    bo_col = np.asarray(out_b, dtype=np.float32).reshape(NCH, P).T
    gamma_col = np.full((P, 1), np.asarray(gamma, dtype=np.float32).reshape(-1)[0],
                        dtype=np.float32)
    # tta[p, (k, b, n)] = text[b, n, k*128+p], fp16 (dtype cast only)
    tta = np.ascontiguousarray(
        text.transpose(2, 0, 1).reshape(NCH, P, B, NT)
        .transpose(1, 0, 2, 3).reshape(P, TTA)).astype(np.float16)
    id8 = np.zeros((P, 8), dtype=np.float32)
    id8[:8, :] = np.eye(8, dtype=np.float32)

    v16 = visual.astype(np.float16)  # (B, C, H, W) -> fp16, dtype cast only
    in_maps = []
    for c in range(B):
        blk = slice(c * P, (c + 1) * P)
        wpb_sb = sb_layout(proj_w[blk].T, ncols=P)
        bp_blk = proj_b[blk].reshape(P, 1)
        colblob = np.ascontiguousarray(
            np.concatenate([bv_col, bo_col, bp_blk, gamma_col, id8],
                           axis=1), dtype=np.float32)
        vis_c = np.ascontiguousarray(
            v16[:, blk, :].reshape(B * P, HW))
        in_maps.append({
            "vis": vis_c,
            "wv_sb": wv_sb, "wo_sb": wo_sb, "wpb_sb": wpb_sb,
            "textblob": tta, "colblob": colblob,
        })

    nc = _get_nc()
    trace = os.environ.get("BASS_KERNEL_TRACE", "") == "1"
    if trace:
        _install_ntff_hook()
    try:
        res = run_bass_kernel_spmd(nc, in_maps, core_ids=list(range(B)), trace=trace)
    except Exception:
        res = run_bass_kernel_spmd(nc, in_maps, core_ids=list(range(B)), trace=trace)
    if trace:
        _NC_CACHE["last_results"] = res

    out = np.empty((B, C, HW), dtype=np.float32)
    for c in range(B):
        out[:, c * P : (c + 1) * P, :] = res.results[c]["out"].reshape(B, P, HW)
    return out.reshape(B, C, H, W)


# revision 25
# speedup vs baseline: 1.0523x; 1.0523x over previous
"""Trainium2 Bass kernel for CrossModalAttention2D — channel-sharded variant.

Same math as the batch-parallel version (see kernel docstring history):
out = visual + gamma * p[b],  p = ((t Wv^T + bv) Wo^T + bo) Wp^T + bp.

Sharding: core c handles channel block [128c, 128(c+1)) of ALL 8 batches.
The chain's first two layers (full 1024-wide outputs) are computed on
every core with M=8 batch lanes, but layer 3 only needs the core's 128
output rows of proj_w — the 1 MB fp8 Wp shrinks to 128 KB per core,
cutting per-core HBM traffic from 19.05 to 18.2 MB.
"""

import os
import sys

sys.path.insert(0, "/opt/trn_rl_repo")

import numpy as np

import concourse.bass as bass
import concourse.mybir as mybir
from concourse.tile import TileContext
from concourse.bass_utils import run_bass_kernel_spmd

B, C, H, W, NH, NT = 8, 1024, 64, 64, 16, 8
HW = H * W
P = 128
NCH = C // P
F32 = mybir.dt.float32
F16 = mybir.dt.float16
BF16 = mybir.dt.bfloat16
F8 = mybir.dt.float8e4
WDT = F8
PADW = 16
HALF = C // 2
HC = NCH * C // 2      # half of a full weight matrix's SBUF cols
HCB = NCH * P // 2     # half of the Wp-block's SBUF cols

# textblob (fp16): tta (NCH*B*NT=512 cols) — rides SWDGE's first push so
# the 128 KB stays off the HWDGE heads (weights there instead).
# colblob (fp32, 13 KB): [ bvc (8) | boc (8) | bpb (1) | gamma (1) |
#                          ident8 (8, rows 0-7) ]
TTA = NCH * B * NT
CBW = 2 * NCH + 2 + 8


def _split_waits(nc):
    for fn in nc.m.functions:
        for blk in fn.blocks:
            rebuilt = []
            changed = False
            for inst in blk.instructions:
                si = inst.sync_info
                if si is not None and si.on_wait is not None and len(si.on_wait) > 1:
                    waits = list(si.on_wait)
                    for i, w in enumerate(waits[:-1]):
                        rebuilt.append(
                            mybir.InstNoOp(
                                name=f"{inst.name}-ws{i}",
                                engine=inst.engine,
                                sync_info=mybir.SyncInfo(on_wait=[w], on_update=[]),
                                bass_nofuse=True,
                            )
                        )
                    si.on_wait = [waits[-1]]
                    changed = True
                rebuilt.append(inst)
            if changed:
                blk.instructions = rebuilt


def _build_nc():
    nc = bass.Bass(trn_type="TRN2")

    vis = nc.dram_tensor("vis", [B * P, HW], F16, kind="ExternalInput")
    wv_sb = nc.dram_tensor("wv_sb", [P, NCH * C], WDT, kind="ExternalInput")
    wo_sb = nc.dram_tensor("wo_sb", [P, NCH * C], WDT, kind="ExternalInput")
    wpb_sb = nc.dram_tensor("wpb_sb", [P, NCH * P], WDT, kind="ExternalInput")
    textblob = nc.dram_tensor("textblob", [P, TTA], F16, kind="ExternalInput")
    colblob = nc.dram_tensor("colblob", [P, CBW], F32, kind="ExternalInput")
    out = nc.dram_tensor("out", [B * P, HW], F16, kind="ExternalOutput")

    with TileContext(nc) as tc:
        with (
            tc.tile_pool(name="cols", bufs=1) as cols,
            tc.tile_pool(name="wpool", bufs=6) as wpool,
            tc.tile_pool(name="psum", bufs=NCH, space="PSUM") as psum,
            tc.tile_pool(name="vispool", bufs=B) as vispool,
        ):
            # ---- weight halves lead both HWDGE queues (per-queue delivery
            # is ~1/3 fabric rate; head position = early arrival) ----
            wts = {
                "v": [wpool.tile([P, HC], WDT, tag=f"wtv{h}", bufs=1,
                                 name=f"wtv{h}") for h in range(2)],
                "u": [wpool.tile([P, HC], WDT, tag=f"wtu{h}", bufs=1,
                                 name=f"wtu{h}") for h in range(2)],
                "g": [wpool.tile([P, HCB], WDT, tag=f"wtg{h}", bufs=1,
                                 name=f"wtg{h}") for h in range(2)],
            }
            nc.sync.dma_start(out=wts["v"][0], in_=wv_sb[:, :HC])
            nc.sync.dma_start(out=wts["u"][0], in_=wo_sb[:, :HC])
            nc.sync.dma_start(out=wts["g"][0], in_=wpb_sb[:, :HCB])
            cb = cols.tile([P, CBW], F32, tag="cb")
            nc.scalar.dma_start(out=cb, in_=colblob[:, :])
            nc.scalar.dma_start(out=wts["v"][1], in_=wv_sb[:, HC:])
            nc.scalar.dma_start(out=wts["u"][1], in_=wo_sb[:, HC:])
            nc.scalar.dma_start(out=wts["g"][1], in_=wpb_sb[:, HCB:])
            tbl = cols.tile([P, TTA], F16, tag="tbl")
            nc.gpsimd.dma_start(out=tbl, in_=textblob[:, :])

            tta = tbl.rearrange("p (k b n) -> p (k b) n", k=NCH, b=B, n=NT)
            bvc = cb[:, 0:NCH]
            boc = cb[:, NCH : 2 * NCH]
            bpb = cb[:, 2 * NCH : 2 * NCH + 1]
            gc = cb[:, 2 * NCH + 1 : 2 * NCH + 2]
            id8f = cb[0:8, 2 * NCH + 2 : CBW]
            id8b = cols.tile([8, 8], BF16, tag="id8b")
            nc.vector.tensor_copy(id8b, id8f)

            vts = {m: vispool.tile([P, HW], F16, tag="vt", name=f"vt{m}")
                   for m in range(B)}

            def vload(eng, m):
                eng.dma_start(out=vts[m], in_=vis[m * P : (m + 1) * P, :])

            # SWDGE carries only 2 loads: the HWDGE queues keep 3 loads each
            # so they don't run dry at ~34us while the first stores wait on
            # gp (~36us) — measured as a 2us full-fabric hole otherwise
            for m in (4, 5):
                vload(nc.gpsimd, m)
            vload(nc.sync, 0)
            vload(nc.sync, 2)
            vload(nc.sync, 7)
            vload(nc.scalar, 1)
            vload(nc.scalar, 3)
            vload(nc.scalar, 6)

            # ---- t[b] = mean_n text[b,:,:], all batches, written to the
            # padded stride-16 layout: value (k-chunk, batch) at col 16k+b
            tsum = cols.tile([P, NCH * B], F32, tag="tsum")
            nc.vector.reduce_sum(tsum, tta, axis=mybir.AxisListType.X)
            nc.vector.tensor_scalar_mul(tsum, tsum, 1.0 / NT)
            tb = cols.tile([P, PADW * NCH], WDT, tag="tb")
            nc.vector.tensor_copy(
                tb.rearrange("p (k s) -> p k s", s=PADW)[:, :, 0:B],
                tsum.rearrange("p (k b) -> p k b", b=B))

            # ---- chain with M=8 batch lanes, fp8 DoubleRow ----
            def layer(in_tile, whalves, bias_cols, name):
                last = name == "g"
                FD = P if last else HALF      # rhs free dim per pass
                nh = 1 if last else 2
                psr = [psum.tile([B, FD], F32, tag="ps", name=f"psr_{name}{h}")
                       for h in range(nh)]
                kw = (NCH * P // 2) // P if last else (HC // C)  # k per half
                wks = [wh.rearrange("p (k c) -> p k c", k=NCH // 2)
                       for wh in whalves]
                for kp in range(NCH // 2):
                    lhsT = in_tile[:, 2 * PADW * kp : 2 * PADW * (kp + 1)]\
                        .rearrange("p (two s) -> p two s", two=2)[:, :, 0:B]
                    wk = wks[kp // 2]
                    kl = 2 * (kp % 2)
                    for h in range(nh):
                        nc.tensor.matmul(
                            psr[h], lhsT,
                            wk[:, kl : kl + 2, h * FD : (h + 1) * FD],
                            start=(kp == 0),
                            stop=(kp == NCH // 2 - 1),
                            perf_mode=mybir.MatmulPerfMode.DoubleRow,
                        )
                row = cols.tile([B, P if last else C],
                                F32 if last else BF16, tag=f"row{name}")
                for h in range(nh):
                    nc.vector.tensor_copy(row[:, h * FD : (h + 1) * FD], psr[h])
                if last:
                    pc = psum.tile([P, B], F32, tag="ps", name="psT_g")
                    nc.tensor.transpose(pc, row, id8f)
                    gp_t = cols.tile([P, B], F32, tag="gp")
                    # gp[:, b] = (p_blk + bp_blk) * gamma, all batches at once
                    nc.vector.tensor_scalar(
                        gp_t, pc, bpb[:, 0:1], gc[:, 0:1],
                        op0=mybir.AluOpType.add, op1=mybir.AluOpType.mult,
                    )
                    return gp_t
                out_tile = cols.tile([P, PADW * NCH], WDT, tag=f"oc{name}")
                for mo in range(NCH):
                    pc = psum.tile([P, B], BF16, tag="ps", name=f"psT_{name}{mo}")
                    nc.tensor.transpose(
                        pc, row[:, mo * P : (mo + 1) * P], id8b)
                    nc.vector.tensor_scalar_add(
                        out_tile[:, PADW * mo : PADW * mo + B], pc,
                        bias_cols[:, mo : mo + 1])
                return out_tile

            vtile = layer(tb, wts["v"], bvc, "v")
            utile = layer(vtile, wts["u"], boc, "u")
            gp = layer(utile, wts["g"], None, "g")

            ADD_ORDER = [0, 1, 4, 2, 3, 5, 6, 7]
            for m in ADD_ORDER:
                nc.vector.tensor_scalar_add(vts[m], vts[m], gp[:, m : m + 1])

            for m in (0, 2, 5, 7):
                if m == 7:
                    nc.sync.dma_start(out=out[m * P : (m + 1) * P, : HW // 2],
                                      in_=vts[m][:, : HW // 2])
                else:
                    nc.sync.dma_start(out=out[m * P : (m + 1) * P, :],
                                      in_=vts[m])
            for m in (1, 4, 3, 6, 7):
                if m == 7:
                    nc.scalar.dma_start(out=out[m * P : (m + 1) * P, HW // 2 :],
                                        in_=vts[m][:, HW // 2 :])
                else:
                    nc.scalar.dma_start(out=out[m * P : (m + 1) * P, :],
                                        in_=vts[m])

    _split_waits(nc)
    return nc


def _install_ntff_hook():
    try:
        from antenv.axon_hooks import get_axon_ntff_profile_hook  # noqa: F401
        return
    except ImportError:
        pass
    import contextlib
    import ctypes
    import types

    so_path = "/opt/axon/libaxon_pjrt.so"
    if not os.path.exists(so_path):
        return
    lib = ctypes.CDLL(so_path)
    if not hasattr(lib, "axon_start_nrt_profile"):
        return
    lib.axon_start_nrt_profile.argtypes = [
        ctypes.POINTER(ctypes.c_int64), ctypes.c_size_t,
    ]
    lib.axon_start_nrt_profile.restype = ctypes.c_int64
    lib.axon_stop_nrt_profile.argtypes = [ctypes.c_char_p]
    lib.axon_stop_nrt_profile.restype = ctypes.c_int64

    @contextlib.contextmanager
    def _hook(output_dir, device_ids):
        import jax

        jax.devices()
        if device_ids:
            ids = (ctypes.c_int64 * len(device_ids))(*device_ids)
            rc = lib.axon_start_nrt_profile(ids, len(device_ids))
        else:
            rc = lib.axon_start_nrt_profile(None, 0)
        if rc != 0:
            raise RuntimeError(f"axon_start_nrt_profile rc={rc}")
        try:
            yield
        finally:
            n = lib.axon_stop_nrt_profile(str(output_dir).encode())
            print(f"ntff profile: {n} file(s) written to {output_dir}")

    import antenv

    mod = types.ModuleType("antenv.axon_hooks")
    mod.get_axon_ntff_profile_hook = lambda: _hook
    mod.set_axon_ntff_profile_hook = lambda h: None
    sys.modules["antenv.axon_hooks"] = mod
    antenv.axon_hooks = mod


_NC_CACHE = {}


def _get_nc():
    if "nc" not in _NC_CACHE:
        _NC_CACHE["nc"] = _build_nc()
    return _NC_CACHE["nc"]


def kernel(visual, text, in_proj_w, in_proj_b, out_w, out_b, ln_w, ln_b,
           proj_w, proj_b, gamma):
    visual = np.asarray(visual, dtype=np.float32)
    text = np.asarray(text, dtype=np.float32)
    in_proj_w = np.asarray(in_proj_w, dtype=np.float32)
    in_proj_b = np.asarray(in_proj_b, dtype=np.float32)
    proj_w = np.asarray(proj_w, dtype=np.float32)
    proj_b = np.asarray(proj_b, dtype=np.float32)

    # host-side input marshalling (layout/dtype only, no math)
    import ml_dtypes

    wdt = ml_dtypes.float8_e4m3fn

    def sb_layout(wT, ncols=C):
        return np.ascontiguousarray(
            wT.reshape(NCH, P, ncols).transpose(1, 0, 2).reshape(P, NCH * ncols)
        ).astype(wdt)

    wv_sb = sb_layout(in_proj_w[2 * C : 3 * C].T)
    wo_sb = sb_layout(np.asarray(out_w, dtype=np.float32).T)

    bv_col = in_proj_b[2 * C : 3 * C].reshape(NCH, P).T
    bo_col = np.asarray(out_b, dtype=np.float32).reshape(NCH, P).T
    gamma_col = np.full((P, 1), np.asarray(gamma, dtype=np.float32).reshape(-1)[0],
                        dtype=np.float32)
    # tta[p, (k, b, n)] = text[b, n, k*128+p], fp16 (dtype cast only)
    tta = np.ascontiguousarray(
        text.transpose(2, 0, 1).reshape(NCH, P, B, NT)
        .transpose(1, 0, 2, 3).reshape(P, TTA)).astype(np.float16)
    id8 = np.zeros((P, 8), dtype=np.float32)
    id8[:8, :] = np.eye(8, dtype=np.float32)

    v16 = visual.astype(np.float16)  # (B, C, H, W) -> fp16, dtype cast only
    in_maps = []
    for c in range(B):
        blk = slice(c * P, (c + 1) * P)
        wpb_sb = sb_layout(proj_w[blk].T, ncols=P)
        bp_blk = proj_b[blk].reshape(P, 1)
        colblob = np.ascontiguousarray(
            np.concatenate([bv_col, bo_col, bp_blk, gamma_col, id8],
                           axis=1), dtype=np.float32)
        vis_c = np.ascontiguousarray(
            v16[:, blk, :].reshape(B * P, HW))
        in_maps.append({
            "vis": vis_c,
            "wv_sb": wv_sb, "wo_sb": wo_sb, "wpb_sb": wpb_sb,
            "textblob": tta, "colblob": colblob,
        })

    nc = _get_nc()
    trace = os.environ.get("BASS_KERNEL_TRACE", "") == "1"
    if trace:
        _install_ntff_hook()
    try:
        res = run_bass_kernel_spmd(nc, in_maps, core_ids=list(range(B)), trace=trace)
    except Exception:
        res = run_bass_kernel_spmd(nc, in_maps, core_ids=list(range(B)), trace=trace)
    if trace:
        _NC_CACHE["last_results"] = res

    out = np.empty((B, C, HW), dtype=np.float32)
    for c in range(B):
        out[:, c * P : (c + 1) * P, :] = res.results[c]["out"].reshape(B, P, HW)
    return out.reshape(B, C, H, W)


# revision 27
# speedup vs baseline: 1.1062x; 1.0511x over previous
"""Trainium2 Bass kernel for CrossModalAttention2D — channel-sharded variant.

Same math as the batch-parallel version (see kernel docstring history):
out = visual + gamma * p[b],  p = ((t Wv^T + bv) Wo^T + bo) Wp^T + bp.

Sharding: core c handles channel block [128c, 128(c+1)) of ALL 8 batches.
The chain's first two layers (full 1024-wide outputs) are computed on
every core with M=8 batch lanes, but layer 3 only needs the core's 128
output rows of proj_w — the 1 MB fp8 Wp shrinks to 128 KB per core,
cutting per-core HBM traffic from 19.05 to 18.2 MB.
"""

import os
import sys

sys.path.insert(0, "/opt/trn_rl_repo")

import numpy as np

import concourse.bass as bass
import concourse.mybir as mybir
from concourse.tile import TileContext
from concourse.bass_utils import run_bass_kernel_spmd

B, C, H, W, NH, NT = 8, 1024, 64, 64, 16, 8
HW = H * W
P = 128
NCH = C // P
F32 = mybir.dt.float32
F16 = mybir.dt.float16
BF16 = mybir.dt.bfloat16
F8 = mybir.dt.float8e4
WDT = F8
PADW = 16
HALF = C // 2
HC = NCH * C // 2      # half of a full weight matrix's SBUF cols
HCB = NCH * P // 2     # half of the Wp-block's SBUF cols

# textblob (fp16): tta (NCH*B*NT=512 cols) — rides SWDGE's first push so
# the 128 KB stays off the HWDGE heads (weights there instead).
# colblob (fp32, 13 KB): [ bvc (8) | boc (8) | bpb (1) | gamma (1) |
#                          ident8 (8, rows 0-7) ]
TTA = NCH * B * NT
CBW = 2 * NCH + 2 + 8


def _split_waits(nc):
    for fn in nc.m.functions:
        for blk in fn.blocks:
            rebuilt = []
            changed = False
            for inst in blk.instructions:
                si = inst.sync_info
                if si is not None and si.on_wait is not None and len(si.on_wait) > 1:
                    waits = list(si.on_wait)
                    for i, w in enumerate(waits[:-1]):
                        rebuilt.append(
                            mybir.InstNoOp(
                                name=f"{inst.name}-ws{i}",
                                engine=inst.engine,
                                sync_info=mybir.SyncInfo(on_wait=[w], on_update=[]),
                                bass_nofuse=True,
                            )
                        )
                    si.on_wait = [waits[-1]]
                    changed = True
                rebuilt.append(inst)
            if changed:
                blk.instructions = rebuilt


def _build_nc():
    nc = bass.Bass(trn_type="TRN2")

    vis = nc.dram_tensor("vis", [B * P, HW], F16, kind="ExternalInput")
    wv_sb = nc.dram_tensor("wv_sb", [P, NCH * C], WDT, kind="ExternalInput")
    wo_sb = nc.dram_tensor("wo_sb", [P, NCH * C], WDT, kind="ExternalInput")
    wpb_sb = nc.dram_tensor("wpb_sb", [P, NCH * P], WDT, kind="ExternalInput")
    textblob = nc.dram_tensor("textblob", [P, TTA], F16, kind="ExternalInput")
    colblob = nc.dram_tensor("colblob", [P, CBW], F32, kind="ExternalInput")
    out = nc.dram_tensor("out", [B * P, HW], F16, kind="ExternalOutput")

    with TileContext(nc) as tc:
        with (
            tc.tile_pool(name="cols", bufs=1) as cols,
            tc.tile_pool(name="wpool", bufs=6) as wpool,
            tc.tile_pool(name="psum", bufs=NCH, space="PSUM") as psum,
            tc.tile_pool(name="vispool", bufs=B) as vispool,
        ):
            # ---- weight halves lead both HWDGE queues (per-queue delivery
            # is ~1/3 fabric rate; head position = early arrival) ----
            wts = {
                "v": [wpool.tile([P, HC], WDT, tag=f"wtv{h}", bufs=1,
                                 name=f"wtv{h}") for h in range(2)],
                "u": [wpool.tile([P, HC], WDT, tag=f"wtu{h}", bufs=1,
                                 name=f"wtu{h}") for h in range(2)],
                "g": [wpool.tile([P, HCB], WDT, tag=f"wtg{h}", bufs=1,
                                 name=f"wtg{h}") for h in range(2)],
            }
            # one visual load ahead of sync's weights: fills the 4.5-10us
            # ramp (only ~2MB of head weights otherwise) with real flood
            # bytes; layer 1 is gated by the scalar-side half anyway, so
            # gp timing — which has no slack — is unchanged
            vts = {m: vispool.tile([P, HW], F16, tag="vt", name=f"vt{m}")
                   for m in range(B)}
            nc.sync.dma_start(out=vts[0], in_=vis[0:P, :])
            nc.sync.dma_start(out=wts["v"][0], in_=wv_sb[:, :HC])
            nc.sync.dma_start(out=wts["u"][0], in_=wo_sb[:, :HC])
            nc.sync.dma_start(out=wts["g"][0], in_=wpb_sb[:, :HCB])
            cb = cols.tile([P, CBW], F32, tag="cb")
            nc.scalar.dma_start(out=cb, in_=colblob[:, :])
            nc.scalar.dma_start(out=wts["v"][1], in_=wv_sb[:, HC:])
            nc.scalar.dma_start(out=wts["u"][1], in_=wo_sb[:, HC:])
            nc.scalar.dma_start(out=wts["g"][1], in_=wpb_sb[:, HCB:])
            tbl = cols.tile([P, TTA], F16, tag="tbl")
            nc.gpsimd.dma_start(out=tbl, in_=textblob[:, :])

            tta = tbl.rearrange("p (k b n) -> p (k b) n", k=NCH, b=B, n=NT)
            bvc = cb[:, 0:NCH]
            boc = cb[:, NCH : 2 * NCH]
            bpb = cb[:, 2 * NCH : 2 * NCH + 1]
            gc = cb[:, 2 * NCH + 1 : 2 * NCH + 2]
            id8f = cb[0:8, 2 * NCH + 2 : CBW]
            id8b = cols.tile([8, 8], BF16, tag="id8b")
            nc.vector.tensor_copy(id8b, id8f)

            def vload(eng, m):
                eng.dma_start(out=vts[m], in_=vis[m * P : (m + 1) * P, :])

            # SWDGE carries only 2 loads: the HWDGE queues keep 3 loads each
            # so they don't run dry at ~34us while the first stores wait on
            # gp (~36us) — measured as a 2us full-fabric hole otherwise
            for m in (4, 5):
                vload(nc.gpsimd, m)
            vload(nc.sync, 2)
            vload(nc.sync, 7)
            vload(nc.scalar, 1)
            vload(nc.scalar, 3)
            vload(nc.scalar, 6)

            # ---- t[b] = mean_n text[b,:,:], all batches, written to the
            # padded stride-16 layout: value (k-chunk, batch) at col 16k+b
            tsum = cols.tile([P, NCH * B], F32, tag="tsum")
            nc.vector.reduce_sum(tsum, tta, axis=mybir.AxisListType.X)
            nc.vector.tensor_scalar_mul(tsum, tsum, 1.0 / NT)
            tb = cols.tile([P, PADW * NCH], WDT, tag="tb")
            nc.vector.tensor_copy(
                tb.rearrange("p (k s) -> p k s", s=PADW)[:, :, 0:B],
                tsum.rearrange("p (k b) -> p k b", b=B))

            # ---- chain with M=8 batch lanes, fp8 DoubleRow ----
            def layer(in_tile, whalves, bias_cols, name):
                last = name == "g"
                FD = P if last else HALF      # rhs free dim per pass
                nh = 1 if last else 2
                psr = [psum.tile([B, FD], F32, tag="ps", name=f"psr_{name}{h}")
                       for h in range(nh)]
                kw = (NCH * P // 2) // P if last else (HC // C)  # k per half
                wks = [wh.rearrange("p (k c) -> p k c", k=NCH // 2)
                       for wh in whalves]
                for kp in range(NCH // 2):
                    lhsT = in_tile[:, 2 * PADW * kp : 2 * PADW * (kp + 1)]\
                        .rearrange("p (two s) -> p two s", two=2)[:, :, 0:B]
                    wk = wks[kp // 2]
                    kl = 2 * (kp % 2)
                    for h in range(nh):
                        nc.tensor.matmul(
                            psr[h], lhsT,
                            wk[:, kl : kl + 2, h * FD : (h + 1) * FD],
                            start=(kp == 0),
                            stop=(kp == NCH // 2 - 1),
                            perf_mode=mybir.MatmulPerfMode.DoubleRow,
                        )
                row = cols.tile([B, P if last else C],
                                F32 if last else BF16, tag=f"row{name}")
                for h in range(nh):
                    nc.vector.tensor_copy(row[:, h * FD : (h + 1) * FD], psr[h])
                if last:
                    pc = psum.tile([P, B], F32, tag="ps", name="psT_g")
                    nc.tensor.transpose(pc, row, id8f)
                    gp_t = cols.tile([P, B], F32, tag="gp")
                    # gp[:, b] = (p_blk + bp_blk) * gamma, all batches at once
                    nc.vector.tensor_scalar(
                        gp_t, pc, bpb[:, 0:1], gc[:, 0:1],
                        op0=mybir.AluOpType.add, op1=mybir.AluOpType.mult,
                    )
                    return gp_t
                out_tile = cols.tile([P, PADW * NCH], WDT, tag=f"oc{name}")
                for mo in range(NCH):
                    pc = psum.tile([P, B], BF16, tag="ps", name=f"psT_{name}{mo}")
                    nc.tensor.transpose(
                        pc, row[:, mo * P : (mo + 1) * P], id8b)
                    nc.vector.tensor_scalar_add(
                        out_tile[:, PADW * mo : PADW * mo + B], pc,
                        bias_cols[:, mo : mo + 1])
                return out_tile

            vtile = layer(tb, wts["v"], bvc, "v")
            utile = layer(vtile, wts["u"], boc, "u")
            gp = layer(utile, wts["g"], None, "g")

            ADD_ORDER = [0, 1, 4, 2, 3, 5, 6, 7]
            for m in ADD_ORDER:
                nc.vector.tensor_scalar_add(vts[m], vts[m], gp[:, m : m + 1])

            for m in (0, 2, 5, 7):
                if m == 7:
                    nc.sync.dma_start(out=out[m * P : (m + 1) * P, : HW // 2],
                                      in_=vts[m][:, : HW // 2])
                else:
                    nc.sync.dma_start(out=out[m * P : (m + 1) * P, :],
                                      in_=vts[m])
            for m in (1, 4, 3, 6, 7):
                if m == 7:
                    nc.scalar.dma_start(out=out[m * P : (m + 1) * P, HW // 2 :],
                                        in_=vts[m][:, HW // 2 :])
                else:
                    nc.scalar.dma_start(out=out[m * P : (m + 1) * P, :],
                                        in_=vts[m])

    _split_waits(nc)
    return nc


def _install_ntff_hook():
    try:
        from antenv.axon_hooks import get_axon_ntff_profile_hook  # noqa: F401
        return
    except ImportError:
        pass
    import contextlib
    import ctypes
    import types

    so_path = "/opt/axon/libaxon_pjrt.so"
    if not os.path.exists(so_path):
        return
    lib = ctypes.CDLL(so_path)
    if not hasattr(lib, "axon_start_nrt_profile"):
        return
    lib.axon_start_nrt_profile.argtypes = [
        ctypes.POINTER(ctypes.c_int64), ctypes.c_size_t,
    ]
    lib.axon_start_nrt_profile.restype = ctypes.c_int64
    lib.axon_stop_nrt_profile.argtypes = [ctypes.c_char_p]
    lib.axon_stop_nrt_profile.restype = ctypes.c_int64

    @contextlib.contextmanager
    def _hook(output_dir, device_ids):
        import jax

        jax.devices()
        if device_ids:
            ids = (ctypes.c_int64 * len(device_ids))(*device_ids)
            rc = lib.axon_start_nrt_profile(ids, len(device_ids))
        else:
            rc = lib.axon_start_nrt_profile(None, 0)
        if rc != 0:
            raise RuntimeError(f"axon_start_nrt_profile rc={rc}")
        try:
            yield
        finally:
            n = lib.axon_stop_nrt_profile(str(output_dir).encode())
            print(f"ntff profile: {n} file(s) written to {output_dir}")

    import antenv

    mod = types.ModuleType("antenv.axon_hooks")
    mod.get_axon_ntff_profile_hook = lambda: _hook
    mod.set_axon_ntff_profile_hook = lambda h: None
    sys.modules["antenv.axon_hooks"] = mod
    antenv.axon_hooks = mod


_NC_CACHE = {}


def _get_nc():
    if "nc" not in _NC_CACHE:
        _NC_CACHE["nc"] = _build_nc()
    return _NC_CACHE["nc"]


def kernel(visual, text, in_proj_w, in_proj_b, out_w, out_b, ln_w, ln_b,
           proj_w, proj_b, gamma):
    visual = np.asarray(visual, dtype=np.float32)
    text = np.asarray(text, dtype=np.float32)
    in_proj_w = np.asarray(in_proj_w, dtype=np.float32)
    in_proj_b = np.asarray(in_proj_b, dtype=np.float32)
    proj_w = np.asarray(proj_w, dtype=np.float32)
    proj_b = np.asarray(proj_b, dtype=np.float32)

    # host-side input marshalling (layout/dtype only, no math)
    import ml_dtypes

    wdt = ml_dtypes.float8_e4m3fn

    def sb_layout(wT, ncols=C):
        return np.ascontiguousarray(
            wT.reshape(NCH, P, ncols).transpose(1, 0, 2).reshape(P, NCH * ncols)
        ).astype(wdt)

    wv_sb = sb_layout(in_proj_w[2 * C : 3 * C].T)
    wo_sb = sb_layout(np.asarray(out_w, dtype=np.float32).T)

    bv_col = in_proj_b[2 * C : 3 * C].reshape(NCH, P).T
    bo_col = np.asarray(out_b, dtype=np.float32).reshape(NCH, P).T
    gamma_col = np.full((P, 1), np.asarray(gamma, dtype=np.float32).reshape(-1)[0],
                        dtype=np.float32)
    # tta[p, (k, b, n)] = text[b, n, k*128+p], fp16 (dtype cast only)
    tta = np.ascontiguousarray(
        text.transpose(2, 0, 1).reshape(NCH, P, B, NT)
        .transpose(1, 0, 2, 3).reshape(P, TTA)).astype(np.float16)
    id8 = np.zeros((P, 8), dtype=np.float32)
    id8[:8, :] = np.eye(8, dtype=np.float32)

    v16 = visual.astype(np.float16)  # (B, C, H, W) -> fp16, dtype cast only
    in_maps = []
    for c in range(B):
        blk = slice(c * P, (c + 1) * P)
        wpb_sb = sb_layout(proj_w[blk].T, ncols=P)
        bp_blk = proj_b[blk].reshape(P, 1)
        colblob = np.ascontiguousarray(
            np.concatenate([bv_col, bo_col, bp_blk, gamma_col, id8],
                           axis=1), dtype=np.float32)
        vis_c = np.ascontiguousarray(
            v16[:, blk, :].reshape(B * P, HW))
        in_maps.append({
            "vis": vis_c,
            "wv_sb": wv_sb, "wo_sb": wo_sb, "wpb_sb": wpb_sb,
            "textblob": tta, "colblob": colblob,
        })

    nc = _get_nc()
    trace = os.environ.get("BASS_KERNEL_TRACE", "") == "1"
    if trace:
        _install_ntff_hook()
    try:
        res = run_bass_kernel_spmd(nc, in_maps, core_ids=list(range(B)), trace=trace)
    except Exception:
        res = run_bass_kernel_spmd(nc, in_maps, core_ids=list(range(B)), trace=trace)
    if trace:
        _NC_CACHE["last_results"] = res

    out = np.empty((B, C, HW), dtype=np.float32)
    for c in range(B):
        out[:, c * P : (c + 1) * P, :] = res.results[c]["out"].reshape(B, P, HW)
    return out.reshape(B, C, H, W)


# revision 29
# speedup vs baseline: 1.1887x; 1.0747x over previous
"""Trainium2 Bass kernel for CrossModalAttention2D — channel-sharded variant.

Same math as the batch-parallel version (see kernel docstring history):
out = visual + gamma * p[b],  p = ((t Wv^T + bv) Wo^T + bo) Wp^T + bp.

Sharding: core c handles channel block [128c, 128(c+1)) of ALL 8 batches.
The chain's first two layers (full 1024-wide outputs) are computed on
every core with M=8 batch lanes, but layer 3 only needs the core's 128
output rows of proj_w — the 1 MB fp8 Wp shrinks to 128 KB per core,
cutting per-core HBM traffic from 19.05 to 18.2 MB.
"""

import os
import sys

sys.path.insert(0, "/opt/trn_rl_repo")

import numpy as np

import concourse.bass as bass
import concourse.mybir as mybir
from concourse.tile import TileContext
from concourse.bass_utils import run_bass_kernel_spmd

B, C, H, W, NH, NT = 8, 1024, 64, 64, 16, 8
HW = H * W
P = 128
NCH = C // P
F32 = mybir.dt.float32
F16 = mybir.dt.float16
BF16 = mybir.dt.bfloat16
F8 = mybir.dt.float8e4
WDT = F8
PADW = 16
HALF = C // 2
HC = NCH * C // 2      # half of a full weight matrix's SBUF cols
HCB = NCH * P // 2     # half of the Wp-block's SBUF cols

# textblob (fp16): tta (NCH*B*NT=512 cols) — rides SWDGE's first push so
# the 128 KB stays off the HWDGE heads (weights there instead).
# colblob (fp32, 13 KB): [ bvc (8) | boc (8) | bpb (1) | gamma (1) |
#                          ident8 (8, rows 0-7) ]
TTA = NCH * B * NT
CBW = 2 * NCH + 2 + 8


def _split_waits(nc):
    for fn in nc.m.functions:
        for blk in fn.blocks:
            rebuilt = []
            changed = False
            for inst in blk.instructions:
                si = inst.sync_info
                if si is not None and si.on_wait is not None and len(si.on_wait) > 1:
                    waits = list(si.on_wait)
                    for i, w in enumerate(waits[:-1]):
                        rebuilt.append(
                            mybir.InstNoOp(
                                name=f"{inst.name}-ws{i}",
                                engine=inst.engine,
                                sync_info=mybir.SyncInfo(on_wait=[w], on_update=[]),
                                bass_nofuse=True,
                            )
                        )
                    si.on_wait = [waits[-1]]
                    changed = True
                rebuilt.append(inst)
            if changed:
                blk.instructions = rebuilt


def _build_nc():
    nc = bass.Bass(trn_type="TRN2")

    vis = nc.dram_tensor("vis", [B * P, HW], F16, kind="ExternalInput")
    wv_sb = nc.dram_tensor("wv_sb", [P, NCH * C], WDT, kind="ExternalInput")
    wo_sb = nc.dram_tensor("wo_sb", [P, NCH * C], WDT, kind="ExternalInput")
    wpb_sb = nc.dram_tensor("wpb_sb", [P, NCH * P], WDT, kind="ExternalInput")
    textblob = nc.dram_tensor("textblob", [P, TTA], F16, kind="ExternalInput")
    colblob = nc.dram_tensor("colblob", [P, CBW], F32, kind="ExternalInput")
    out = nc.dram_tensor("out", [B * P, HW], F16, kind="ExternalOutput")

    with TileContext(nc) as tc:
        with (
            tc.tile_pool(name="cols", bufs=1) as cols,
            tc.tile_pool(name="wpool", bufs=6) as wpool,
            tc.tile_pool(name="psum", bufs=NCH, space="PSUM") as psum,
            tc.tile_pool(name="vispool", bufs=B) as vispool,
        ):
            # ---- weight halves lead both HWDGE queues (per-queue delivery
            # is ~1/3 fabric rate; head position = early arrival) ----
            wts = {
                "v": [wpool.tile([P, HC], WDT, tag=f"wtv{h}", bufs=1,
                                 name=f"wtv{h}") for h in range(2)],
                "u": [wpool.tile([P, HC], WDT, tag=f"wtu{h}", bufs=1,
                                 name=f"wtu{h}") for h in range(2)],
                "g": [wpool.tile([P, HCB], WDT, tag=f"wtg{h}", bufs=1,
                                 name=f"wtg{h}") for h in range(2)],
            }
            # half a visual load ahead of EACH HWDGE queue's weights: fills
            # the 4.5-10us ramp (only ~2MB of head weights otherwise) with
            # real flood bytes on both queues, while the weights land only
            # ~2us later than at absolute head — the chain still finishes
            # with slack before stores are needed
            vts = {m: vispool.tile([P, HW], F16, tag="vt", name=f"vt{m}")
                   for m in range(B)}
            nc.sync.dma_start(out=vts[0][:, : HW // 2], in_=vis[0:P, : HW // 2])
            nc.sync.dma_start(out=wts["v"][0], in_=wv_sb[:, :HC])
            nc.sync.dma_start(out=wts["u"][0], in_=wo_sb[:, :HC])
            nc.sync.dma_start(out=wts["g"][0], in_=wpb_sb[:, :HCB])
            nc.sync.dma_start(out=vts[0][:, HW // 2 :], in_=vis[0:P, HW // 2 :])
            cb = cols.tile([P, CBW], F32, tag="cb")
            nc.scalar.dma_start(out=vts[1][:, : HW // 2], in_=vis[P : 2 * P, : HW // 2])
            nc.scalar.dma_start(out=cb, in_=colblob[:, :])
            nc.scalar.dma_start(out=wts["v"][1], in_=wv_sb[:, HC:])
            nc.scalar.dma_start(out=wts["u"][1], in_=wo_sb[:, HC:])
            nc.scalar.dma_start(out=wts["g"][1], in_=wpb_sb[:, HCB:])
            nc.scalar.dma_start(out=vts[1][:, HW // 2 :], in_=vis[P : 2 * P, HW // 2 :])
            tbl = cols.tile([P, TTA], F16, tag="tbl")
            nc.gpsimd.dma_start(out=tbl, in_=textblob[:, :])

            tta = tbl.rearrange("p (k b n) -> p (k b) n", k=NCH, b=B, n=NT)
            bvc = cb[:, 0:NCH]
            boc = cb[:, NCH : 2 * NCH]
            bpb = cb[:, 2 * NCH : 2 * NCH + 1]
            gc = cb[:, 2 * NCH + 1 : 2 * NCH + 2]
            id8f = cb[0:8, 2 * NCH + 2 : CBW]
            id8b = cols.tile([8, 8], BF16, tag="id8b")
            nc.vector.tensor_copy(id8b, id8f)

            def vload(eng, m):
                eng.dma_start(out=vts[m], in_=vis[m * P : (m + 1) * P, :])

            # SWDGE carries only 2 loads: the HWDGE queues keep 3 loads each
            # so they don't run dry at ~34us while the first stores wait on
            # gp (~36us) — measured as a 2us full-fabric hole otherwise
            for m in (4, 5):
                vload(nc.gpsimd, m)
            vload(nc.sync, 2)
            vload(nc.sync, 7)
            vload(nc.scalar, 3)
            vload(nc.scalar, 6)

            # ---- t[b] = mean_n text[b,:,:], all batches, written to the
            # padded stride-16 layout: value (k-chunk, batch) at col 16k+b
            tsum = cols.tile([P, NCH * B], F32, tag="tsum")
            nc.vector.reduce_sum(tsum, tta, axis=mybir.AxisListType.X)
            nc.vector.tensor_scalar_mul(tsum, tsum, 1.0 / NT)
            tb = cols.tile([P, PADW * NCH], WDT, tag="tb")
            nc.vector.tensor_copy(
                tb.rearrange("p (k s) -> p k s", s=PADW)[:, :, 0:B],
                tsum.rearrange("p (k b) -> p k b", b=B))

            # ---- chain with M=8 batch lanes, fp8 DoubleRow ----
            def layer(in_tile, whalves, bias_cols, name):
                last = name == "g"
                FD = P if last else HALF      # rhs free dim per pass
                nh = 1 if last else 2
                psr = [psum.tile([B, FD], F32, tag="ps", name=f"psr_{name}{h}")
                       for h in range(nh)]
                kw = (NCH * P // 2) // P if last else (HC // C)  # k per half
                wks = [wh.rearrange("p (k c) -> p k c", k=NCH // 2)
                       for wh in whalves]
                for kp in range(NCH // 2):
                    lhsT = in_tile[:, 2 * PADW * kp : 2 * PADW * (kp + 1)]\
                        .rearrange("p (two s) -> p two s", two=2)[:, :, 0:B]
                    wk = wks[kp // 2]
                    kl = 2 * (kp % 2)
                    for h in range(nh):
                        nc.tensor.matmul(
                            psr[h], lhsT,
                            wk[:, kl : kl + 2, h * FD : (h + 1) * FD],
                            start=(kp == 0),
                            stop=(kp == NCH // 2 - 1),
                            perf_mode=mybir.MatmulPerfMode.DoubleRow,
                        )
                row = cols.tile([B, P if last else C],
                                F32 if last else BF16, tag=f"row{name}")
                for h in range(nh):
                    nc.vector.tensor_copy(row[:, h * FD : (h + 1) * FD], psr[h])
                if last:
                    pc = psum.tile([P, B], F32, tag="ps", name="psT_g")
                    nc.tensor.transpose(pc, row, id8f)
                    gp_t = cols.tile([P, B], F32, tag="gp")
                    # gp[:, b] = (p_blk + bp_blk) * gamma, all batches at once
                    nc.vector.tensor_scalar(
                        gp_t, pc, bpb[:, 0:1], gc[:, 0:1],
                        op0=mybir.AluOpType.add, op1=mybir.AluOpType.mult,
                    )
                    return gp_t
                out_tile = cols.tile([P, PADW * NCH], WDT, tag=f"oc{name}")
                for mo in range(NCH):
                    pc = psum.tile([P, B], BF16, tag="ps", name=f"psT_{name}{mo}")
                    nc.tensor.transpose(
                        pc, row[:, mo * P : (mo + 1) * P], id8b)
                    nc.vector.tensor_scalar_add(
                        out_tile[:, PADW * mo : PADW * mo + B], pc,
                        bias_cols[:, mo : mo + 1])
                return out_tile

            vtile = layer(tb, wts["v"], bvc, "v")
            utile = layer(vtile, wts["u"], boc, "u")
            gp = layer(utile, wts["g"], None, "g")

            ADD_ORDER = [0, 1, 4, 2, 3, 5, 6, 7]
            for m in ADD_ORDER:
                nc.vector.tensor_scalar_add(vts[m], vts[m], gp[:, m : m + 1])

            for m in (0, 2, 5, 7):
                if m == 7:
                    nc.sync.dma_start(out=out[m * P : (m + 1) * P, : HW // 2],
                                      in_=vts[m][:, : HW // 2])
                else:
                    nc.sync.dma_start(out=out[m * P : (m + 1) * P, :],
                                      in_=vts[m])
            for m in (1, 4, 3, 6, 7):
                if m == 7:
                    nc.scalar.dma_start(out=out[m * P : (m + 1) * P, HW // 2 :],
                                        in_=vts[m][:, HW // 2 :])
                else:
                    nc.scalar.dma_start(out=out[m * P : (m + 1) * P, :],
                                        in_=vts[m])

    _split_waits(nc)
    return nc


def _install_ntff_hook():
    try:
        from antenv.axon_hooks import get_axon_ntff_profile_hook  # noqa: F401
        return
    except ImportError:
        pass
    import contextlib
    import ctypes
    import types

    so_path = "/opt/axon/libaxon_pjrt.so"
    if not os.path.exists(so_path):
        return
    lib = ctypes.CDLL(so_path)
    if not hasattr(lib, "axon_start_nrt_profile"):
        return
    lib.axon_start_nrt_profile.argtypes = [
        ctypes.POINTER(ctypes.c_int64), ctypes.c_size_t,
    ]
    lib.axon_start_nrt_profile.restype = ctypes.c_int64
    lib.axon_stop_nrt_profile.argtypes = [ctypes.c_char_p]
    lib.axon_stop_nrt_profile.restype = ctypes.c_int64

    @contextlib.contextmanager
    def _hook(output_dir, device_ids):
        import jax

        jax.devices()
        if device_ids:
            ids = (ctypes.c_int64 * len(device_ids))(*device_ids)
            rc = lib.axon_start_nrt_profile(ids, len(device_ids))
        else:
            rc = lib.axon_start_nrt_profile(None, 0)
        if rc != 0:
            raise RuntimeError(f"axon_start_nrt_profile rc={rc}")
        try:
            yield
        finally:
            n = lib.axon_stop_nrt_profile(str(output_dir).encode())
            print(f"ntff profile: {n} file(s) written to {output_dir}")

    import antenv

    mod = types.ModuleType("antenv.axon_hooks")
    mod.get_axon_ntff_profile_hook = lambda: _hook
    mod.set_axon_ntff_profile_hook = lambda h: None
    sys.modules["antenv.axon_hooks"] = mod
    antenv.axon_hooks = mod


_NC_CACHE = {}


def _get_nc():
    if "nc" not in _NC_CACHE:
        _NC_CACHE["nc"] = _build_nc()
    return _NC_CACHE["nc"]


def kernel(visual, text, in_proj_w, in_proj_b, out_w, out_b, ln_w, ln_b,
           proj_w, proj_b, gamma):
    visual = np.asarray(visual, dtype=np.float32)
    text = np.asarray(text, dtype=np.float32)
    in_proj_w = np.asarray(in_proj_w, dtype=np.float32)
    in_proj_b = np.asarray(in_proj_b, dtype=np.float32)
    proj_w = np.asarray(proj_w, dtype=np.float32)
    proj_b = np.asarray(proj_b, dtype=np.float32)

    # host-side input marshalling (layout/dtype only, no math)
    import ml_dtypes

    wdt = ml_dtypes.float8_e4m3fn

    def sb_layout(wT, ncols=C):
        return np.ascontiguousarray(
            wT.reshape(NCH, P, ncols).transpose(1, 0, 2).reshape(P, NCH * ncols)
        ).astype(wdt)

    wv_sb = sb_layout(in_proj_w[2 * C : 3 * C].T)
    wo_sb = sb_layout(np.asarray(out_w, dtype=np.float32).T)

    bv_col = in_proj_b[2 * C : 3 * C].reshape(NCH, P).T
    bo_col = np.asarray(out_b, dtype=np.float32).reshape(NCH, P).T
    gamma_col = np.full((P, 1), np.asarray(gamma, dtype=np.float32).reshape(-1)[0],
                        dtype=np.float32)
    # tta[p, (k, b, n)] = text[b, n, k*128+p], fp16 (dtype cast only)
    tta = np.ascontiguousarray(
        text.transpose(2, 0, 1).reshape(NCH, P, B, NT)
        .transpose(1, 0, 2, 3).reshape(P, TTA)).astype(np.float16)
    id8 = np.zeros((P, 8), dtype=np.float32)
    id8[:8, :] = np.eye(8, dtype=np.float32)

    v16 = visual.astype(np.float16)  # (B, C, H, W) -> fp16, dtype cast only
    in_maps = []
    for c in range(B):
        blk = slice(c * P, (c + 1) * P)
        wpb_sb = sb_layout(proj_w[blk].T, ncols=P)
        bp_blk = proj_b[blk].reshape(P, 1)
        colblob = np.ascontiguousarray(
            np.concatenate([bv_col, bo_col, bp_blk, gamma_col, id8],
                           axis=1), dtype=np.float32)
        vis_c = np.ascontiguousarray(
            v16[:, blk, :].reshape(B * P, HW))
        in_maps.append({
            "vis": vis_c,
            "wv_sb": wv_sb, "wo_sb": wo_sb, "wpb_sb": wpb_sb,
            "textblob": tta, "colblob": colblob,
        })

    nc = _get_nc()
    trace = os.environ.get("BASS_KERNEL_TRACE", "") == "1"
    if trace:
        _install_ntff_hook()
    try:
        res = run_bass_kernel_spmd(nc, in_maps, core_ids=list(range(B)), trace=trace)
    except Exception:
        res = run_bass_kernel_spmd(nc, in_maps, core_ids=list(range(B)), trace=trace)
    if trace:
        _NC_CACHE["last_results"] = res

    out = np.empty((B, C, HW), dtype=np.float32)
    for c in range(B):
        out[:, c * P : (c + 1) * P, :] = res.results[c]["out"].reshape(B, P, HW)
    return out.reshape(B, C, H, W)
